# revision 15
# baseline (speedup 1.0000x reference)
"""Trainium2 Bass kernel for nn_GATv2_43499428773954.

Sharding (8 NeuronCores, SPMD):
  - Genes (nodes) are sharded across cores by a degree-sorted snake deal:
    every core owns 625 genes (padded to 640 = 5 tiles of 128). Per-gene
    weights W_in and the flatten-dense W1 are read only by the owning core,
    so the dominant HBM traffic is fully sharded.
  - All heavy tensors are f16 on device (W_in, W1, h, xl table, gather
    payloads, attention elementwise) with fp32 PSUM accumulation and an
    fp32 softmax; this halves HBM traffic and runs the PE/DVE at 2-4x
    the fp32 rate. Tolerance is 2e-2 rel; f16 keeps us ~1e-3.
  - GATv2 edge attention uses a padded-CSR layout: per tile of 128
    destination nodes, K_j slots each (K_j = tile max degree). Source
    features come from dma_gather row-gathers (split over 4 SWDGE queues)
    out of an AllGathered f16 xl table in DRAM; all 5 tiles' gathers are
    issued upfront so the gather stream never idles.
  - att is folded into W_l/W_r columns host-side and the d axis permuted
    so att>0 columns come first; then att . leaky(xl[s]+xr[t]) =
    sum_P max(v,.2v) + sum_N min(v,.2v) -> two in-place DVE ops + reduce.
  - h = relu(einsum('gi,gid->gd')) runs on the PE as one matmul per gene
    PAIR: lhsT = the pair's W_in rows stacked on the contraction axis,
    rhs = a block-diagonal [128,2] x column pair (built host-side), which
    yields hT tiles directly in PSUM.
  - Output MLP: per-gene W1 row-block matvecs accumulate into a [1,128]
    PSUM tile on the PE; AllGather of per-core partials + tiny MLP tail.
"""

import math
import numpy as np
F16 = np.float16

G = 5000
IN = 64
D = 128
E = 240000
C = 8
P = 128
GPC = G // C              # 625
TPC = (GPC + P - 1) // P  # 5
PADG = TPC * P            # 640
SLOPE = 0.2
MASKVAL = -1.0e5
NQ = 4                    # SWDGE queues for gather desc-gen
VCH = 32                  # V-buffer chunk (slots of K) for the e pass


# ----------------------------------------------------------------- host prep

def _host_prep(inputs):
    f32 = np.float32
    ei = np.asarray(inputs["edge_index"])
    loop = np.arange(G, dtype=ei.dtype)
    src = np.concatenate([ei[0], loop]).astype(np.int64)
    dst = np.concatenate([ei[1], loop]).astype(np.int64)
    deg = np.bincount(dst, minlength=G)

    order = np.argsort(-deg, kind="stable")
    perm = np.empty((C, GPC), dtype=np.int64)
    k = np.arange(G) % (2 * C)
    core_of_rank = np.where(k < C, k, 2 * C - 1 - k)
    for c in range(C):
        perm[c] = order[core_of_rank == c]
    newid = np.empty(G, dtype=np.int64)
    for c in range(C):
        newid[perm[c]] = c * PADG + np.arange(GPC)

    K = []
    for j in range(TPC):
        lo, hi = j * P, min((j + 1) * P, GPC)
        mx = max(int(deg[perm[c][lo:hi]].max()) for c in range(C))
        K.append(int(math.ceil(mx / 4) * 4))
    sumK = int(np.sum(K))
    S = sumK * P

    # CSR by destination, sources in new-id space
    order_e = np.argsort(dst, kind="stable")
    src_s = newid[src[order_e]]
    dst_s = dst[order_e]
    starts = np.searchsorted(dst_s, np.arange(G))
    ends = np.searchsorted(dst_s, np.arange(G) + 1)

    srcs_hat = np.zeros((C, S), dtype=np.int16)
    mask = np.zeros((C, P, sumK), dtype=f32)
    for c in range(C):
        off = 0
        koff = 0
        for j in range(TPC):
            Kj = K[j]
            for p in range(P):
                pos = j * P + p
                if pos < GPC:
                    g = perm[c][pos]
                    s0, s1 = starts[g], ends[g]
                    nd = s1 - s0
                    srcs_hat[c, off + np.arange(nd) * P + p] = src_s[s0:s1]
                else:
                    nd = 0
                mask[c, p, koff + nd:koff + Kj] = MASKVAL
            off += Kj * P
            koff += Kj
    mask = np.ascontiguousarray(mask)

    # wrap gather indices: logical idx i -> [i % 16, i // 16], replicated
    # into each Q7 core pair's 16-partition slice
    wrapped = srcs_hat.reshape(C, S // 16, 16).transpose(0, 2, 1)
    idx16 = np.tile(wrapped, (1, 8, 1))

    att = np.asarray(inputs["att"], dtype=f32)
    pi = np.argsort(att <= 0, kind="stable")
    p_pos = int((att > 0).sum())
    W_l = np.asarray(inputs["W_l"], f32)
    b_l = np.asarray(inputs["b_l"], f32)
    W_r = np.asarray(inputs["W_r"], f32)
    b_r = np.asarray(inputs["b_r"], f32)
    # xl/xr path carries a x16 scale for fp16 range; inv_att absorbs it
    wl_eff = (W_l * att[None, :] * 16.0)[:, pi].astype(F16)
    bl_eff = (b_l * att * 16.0)[pi].reshape(1, D).astype(F16)
    wr_eff = (W_r * att[None, :] * 16.0)[:, pi].astype(F16)
    br_eff = (b_r * att * 16.0)[pi].reshape(1, D).astype(F16)
    inv_att = np.tile((1.0 / att / 16.0)[pi][None, :], (P, 1)).astype(f32)
    bias_r = np.tile(np.asarray(inputs["bias"], f32)[pi][None, :], (P, 1))

    x = np.asarray(inputs["x"], f32)
    W_in = np.asarray(inputs["W_in"], f32)
    b_in = np.asarray(inputs["b_in"], f32)
    # phase-A layouts. Pair cc of tile j covers local genes (2cc, 2cc+1);
    # contraction index k = m*64 + i.
    #  w_in_hk[c][j, k, cc, d] = W_in[gene(j, 2cc + k//64), k%64, d]
    #  x_bd8[c][k, (j*16+q)*8 + 2p+m] = x[gene(j, 8q+2p+m), k%64] (k//64==m)
    NQUAD = P // 8  # 16 quads per tile
    w_in_hk = np.zeros((C, TPC, P, 64, D), F16)
    x_bd8 = np.zeros((C, P, TPC * NQUAD * 8), F16)
    b_in_T = np.zeros((C, D, TPC * P), f32)
    for c in range(C):
        xp = np.zeros((PADG, IN), f32)
        xp[:GPC] = x[perm[c]]
        wp = np.zeros((PADG, IN, D), f32)
        wp[:GPC] = W_in[perm[c]]
        bp = np.zeros((PADG, D), f32)
        bp[:GPC] = b_in[perm[c]]
        b_in_T[c] = bp.reshape(TPC * P, D).T
        for j in range(TPC):
            gj = wp[j * P:(j + 1) * P].reshape(64, 2, IN, D)  # [cc, m, i, d]
            # x16 scale for fp16 range; unscaled on-chip in the PSUM->hT copy
            w_in_hk[c, j] = (gj.transpose(1, 2, 0, 3).reshape(P, 64, D)
                             * 16.0).astype(F16)
            for q in range(NQUAD):
                for p4 in range(4):
                    for m in range(2):
                        gl = 8 * q + 2 * p4 + m
                        col = (j * NQUAD + q) * 8 + 2 * p4 + m
                        x_bd8[c, m * 64:(m + 1) * 64, col] = xp[j * P + gl]

    # W1 in [d, (gene, n)] layout: w1_h[c][d, g*128+n] = W1r[gene g][pi[d], n]
    W1r = np.asarray(inputs["W1"], f32).reshape(G, D, D)
    w1_h = np.zeros((C, D, PADG * D), F16)
    for c in range(C):
        t = np.zeros((PADG, D, D), f32)
        # x256 scale for fp16 range; unscaled in the MLP tail
        t[:GPC] = W1r[perm[c]][:, pi, :] * 256.0
        w1_h[c] = t.transpose(1, 0, 2).reshape(D, PADG * D)

    b1c = np.asarray(inputs["b1"], f32).reshape(D, 1).copy()
    w2c = np.asarray(inputs["W2"], f32).reshape(D, 1).copy()
    b2c = np.asarray(inputs["b2"], f32).reshape(1, 1).copy()

    return dict(K=K, sumK=sumK, S=S, p_pos=p_pos, idx16=idx16, mask=mask,
                wl_eff=wl_eff, bl_eff=bl_eff, wr_eff=wr_eff, br_eff=br_eff,
                inv_att=inv_att, bias_r=bias_r, x_bd8=x_bd8, w_in_hk=w_in_hk,
                b_in_T=b_in_T, w1_h=w1_h, b1c=b1c, w2c=w2c, b2c=b2c)


# -------------------------------------------------------------- bass builder

def _build(K, sumK, S, p_pos):
    import concourse.bass as bass
    import concourse.bacc as bacc
    import concourse.tile as tile
    from concourse import mybir
    from contextlib import ExitStack

    f32 = mybir.dt.float32
    f16 = mybir.dt.float16
    i16 = mybir.dt.int16
    Alu = mybir.AluOpType
    Act = mybir.ActivationFunctionType
    AxX = mybir.AxisListType.X

    nc = bacc.Bacc("TRN2", target_bir_lowering=False, debug=False,
                   enable_asserts=False, num_devices=C, num_swdge_queues=NQ)

    din = {}
    def inp(name, shape, dtype=f32):
        din[name] = nc.dram_tensor(name, list(shape), dtype,
                                   kind="ExternalInput")
        return din[name]

    inp("x_bd8", [P, TPC * 16 * 8], f16)
    w_in = inp("w_in", [TPC, P, 64, D], f16)
    inp("b_in_T", [D, TPC * P])
    inp("wl", [D, D], f16)
    inp("wr", [D, D], f16)
    inp("blr", [1, D], f16)
    inp("brr", [1, D], f16)
    inp("ident", [P, P], f16)
    inp("invatt", [P, D])
    inp("biasr", [P, D])
    inp("maskf", [P, sumK])
    inp("idx16", [P, S // 16], i16)
    w1 = inp("w1", [D, PADG * D], f16)
    inp("b1c", [D, 1])
    inp("w2c", [D, 1])
    inp("b2c", [1, 1])
    y_out = nc.dram_tensor("y_out", [1, 1], f32, kind="ExternalOutput")

    xl_local = nc.dram_tensor("xl_local", [PADG, D], f16)
    xl_ag = nc.dram_tensor("xl_ag", [C * PADG, D], f16, addr_space="Shared")
    yp_local = nc.dram_tensor("yp_local", [1, D], f32)
    yp_ag = nc.dram_tensor("yp_ag", [C, D], f32, addr_space="Shared")

    rg = [list(range(C))]

    with tile.TileContext(nc) as tc, ExitStack() as ctx:
        cp = ctx.enter_context(tc.tile_pool(name="const", bufs=1))

        def load(name, shape, dtype=f32):
            t = cp.tile(list(shape), dtype, tag=name)
            nc.sync.dma_start(t[:], din[name].ap())
            return t

        xbd_sb = load("x_bd8", [P, TPC * 16 * 8], f16)
        binT_sb = load("b_in_T", [D, TPC * P])
        wl_sb = load("wl", [D, D], f16)
        wr_sb = load("wr", [D, D], f16)
        blr_sb = load("blr", [1, D], f16)
        brr_sb = load("brr", [1, D], f16)
        id_sb = load("ident", [P, P], f16)
        invatt_sb = load("invatt", [P, D])
        biasr_sb = load("biasr", [P, D])
        mask_sb = load("maskf", [P, sumK])
        idx_sb = load("idx16", [P, S // 16], i16)
        b1_sb = load("b1c", [D, 1])
        w2_sb = load("w2c", [D, 1])
        b2_sb = load("b2c", [1, 1])
        ones_r = cp.tile([1, D], f16, tag="ones_r")
        nc.vector.memset(ones_r[:], 1.0)
        ones_c = cp.tile([D, 1], f32, tag="ones_c")
        nc.vector.memset(ones_c[:], 1.0)

        hT = cp.tile([P, TPC * D], f16, tag="hT")
        xr_sb = cp.tile([P, TPC * D], f16, tag="xr")
        outT = cp.tile([P, TPC * D], f16, tag="outT")
        e_sb = cp.tile([P, sumK], f32, tag="e")
        # one big gather target holding all 5 tiles' gathered xl rows
        Xg_all = cp.tile([P, sumK * D], f16, tag="xg_all")

        psp = ctx.enter_context(tc.tile_pool(name="ps", bufs=4, space="PSUM"))
        stg = ctx.enter_context(tc.tile_pool(name="stg", bufs=2))

        # ---------------- phase A: hT = relu(einsum(x, W_in) + b_in).T
        CB = 32  # pair-chunks per DMA (1MB f16)
        with tc.tile_pool(name="win", bufs=3) as winp:
            for j in range(TPC):
                ph = psp.tile([P, P], f32, tag="ph", bufs=2)
                for c0 in range(0, 64, CB):
                    wt = winp.tile([P, CB * D], f16, tag="wt")
                    srcap = bass.AP(w_in, (j * P * 64 + c0) * D,
                                    [[64 * D, P], [1, CB * D]])
                    nc.sync.dma_start(wt[:], srcap)
                    for cc in range(CB):
                        cgl = c0 + cc
                        kcol = (j * 16 + cgl // 4) * 8 + (cgl % 4) * 2
                        nc.tensor.matmul(
                            ph[:, 2 * cgl:2 * cgl + 2],
                            wt[:, cc * D:(cc + 1) * D],
                            xbd_sb[:, kcol:kcol + 2],
                            start=True, stop=True)
                hsl = hT[:, j * D:(j + 1) * D]
                nc.vector.scalar_tensor_tensor(
                    hsl, ph[:], 1.0 / 16.0, binT_sb[:, j * D:(j + 1) * D],
                    Alu.mult, Alu.add)
                nc.vector.tensor_scalar_max(hsl, hsl, 0.0)
                # phase B for this tile: xr, xl, write xl tile to DRAM
                sl = slice(j * D, (j + 1) * D)
                pr = psp.tile([P, P], f32, tag="ph", bufs=2)
                nc.tensor.matmul(pr[:], hT[:, sl], wr_sb[:],
                                 start=True, stop=False)
                nc.tensor.matmul(pr[:], ones_r[:], brr_sb[:],
                                 start=False, stop=True)
                nc.scalar.copy(xr_sb[:, sl], pr[:])
                pl = psp.tile([P, P], f32, tag="ph", bufs=2)
                nc.tensor.matmul(pl[:], hT[:, sl], wl_sb[:],
                                 start=True, stop=False)
                nc.tensor.matmul(pl[:], ones_r[:], blr_sb[:],
                                 start=False, stop=True)
                xl_tmp = stg.tile([P, P], f16, tag="xl_tmp")
                nc.scalar.copy(xl_tmp[:], pl[:])
                nc.sync.dma_start(xl_local[j * P:(j + 1) * P, :], xl_tmp[:])

        nc.gpsimd.collective_compute(
            "AllGather", Alu.bypass, replica_groups=rg,
            ins=[xl_local.ap()], outs=[xl_ag.ap()])

        # ---------------- phase C prologue: issue ALL tiles' gathers upfront
        koffs = [int(np.sum(K[:j])) for j in range(TPC + 1)]
        for j in range(TPC):
            Kj = K[j]
            off = koffs[j] * P
            base = koffs[j] * D  # column offset into Xg_all
            ksplit = [Kj * q // NQ for q in range(NQ + 1)]
            for q in range(NQ):
                k0, k1 = ksplit[q], ksplit[q + 1]
                if k1 == k0:
                    continue
                n = (k1 - k0) * P
                gq = bass.AP(Xg_all.tensor, Xg_all.offset + base + k0 * D,
                             [Xg_all.ap[0], [D, k1 - k0], [1, D]])
                nc.gpsimd.dma_gather(
                    out_ap=gq, in_ap=xl_ag.ap(),
                    idxs_ap=idx_sb[:, (off + k0 * P) // 16:(off + k1 * P) // 16],
                    num_idxs=n, num_idxs_reg=n, elem_size=D,
                    single_packet=False, queue_num=q)

        # ---------------- phase C+D: attention blocks + W1 matvec
        vp = ctx.enter_context(tc.tile_pool(name="v", bufs=2))
        outp = ctx.enter_context(tc.tile_pool(name="outp", bufs=2))
        smp = ctx.enter_context(tc.tile_pool(name="sm", bufs=4))
        w1p = ctx.enter_context(tc.tile_pool(name="w1p", bufs=14))
        pyp = ctx.enter_context(tc.tile_pool(name="py", bufs=1, space="PSUM"))
        # 4-gene-per-matmul W1 accumulator: quad matmul outT[:,4].T @
        # [W1_g0|..|W1_g3] -> [4, 512]; only the 4 diagonal 128-blocks are
        # meaningful, and they accumulate sum_g out_g @ W1_g across quads.
        psum_y = pyp.tile([4, 4 * D], f32, tag="psum_y")

        GB = 16  # genes per W1 DMA (512KB f16)
        for j in range(TPC):
            Kj = K[j]
            koff = koffs[j]
            base = koff * D
            xg_j = bass.AP(Xg_all.tensor, Xg_all.offset + base,
                           [Xg_all.ap[0], [D, Kj], [1, D]])
            xr_j = xr_sb[:, j * D:(j + 1) * D]

            # e pass in V-chunks of VCH k-slots
            for k0 in range(0, Kj, VCH):
                kw = min(VCH, Kj - k0)
                V = vp.tile([P, VCH * D], f16, tag="v")
                Va = bass.AP(V.tensor, V.offset, [V.ap[0], [D, kw], [1, D]])
                Xc = bass.AP(Xg_all.tensor, Xg_all.offset + base + k0 * D,
                             [Xg_all.ap[0], [D, kw], [1, D]])
                xr_b = bass.AP(xr_j.tensor, xr_j.offset,
                               [xr_j.ap[0], [0, kw], xr_j.ap[1]])
                nc.vector.scalar_tensor_tensor(Va, Xc, 0.0, xr_b,
                                               Alu.bypass, Alu.add)
                VP = bass.AP(V.tensor, V.offset, [V.ap[0], [D, kw], [1, p_pos]])
                VN = bass.AP(V.tensor, V.offset + p_pos,
                             [V.ap[0], [D, kw], [1, D - p_pos]])
                nc.vector.scalar_tensor_tensor(VP, VP, SLOPE, VP,
                                               Alu.mult, Alu.max)
                nc.vector.scalar_tensor_tensor(VN, VN, SLOPE, VN,
                                               Alu.mult, Alu.min)
                e_c = e_sb[:, koff + k0:koff + k0 + kw]
                nc.vector.tensor_reduce(e_c, Va, axis=AxX, op=Alu.add)

            e_sl = e_sb[:, koff:koff + Kj]
            nc.vector.tensor_tensor(e_sl, e_sl,
                                    mask_sb[:, koff:koff + Kj], Alu.add)
            m = smp.tile([P, 1], f32, tag="m")
            nc.vector.tensor_reduce(m[:], e_sl, axis=AxX, op=Alu.max)
            negm = smp.tile([P, 1], f32, tag="negm")
            nc.vector.tensor_scalar_mul(negm[:], m[:], -1.0 / 16.0)
            nc.scalar.activation(e_sl, e_sl, Act.Exp, bias=negm[:],
                                 scale=1.0 / 16.0)
            z = smp.tile([P, 1], f32, tag="z")
            nc.vector.tensor_reduce(z[:], e_sl, axis=AxX, op=Alu.add)
            zr = smp.tile([P, 1], f32, tag="zr")
            nc.vector.reciprocal(zr[:], z[:])
            nc.vector.tensor_scalar_mul(e_sl, e_sl, zr[:])   # alpha
            # alpha * gathered rows, one Act-engine copy per k with the
            # per-partition alpha as the activation scale
            for k in range(Kj):
                xgk = bass.AP(Xg_all.tensor, Xg_all.offset + base + k * D,
                              [Xg_all.ap[0], [1, D]])
                nc.scalar.activation(
                    xgk, xgk, Act.Copy,
                    scale=e_sb[:, koff + k:koff + k + 1])
            # aggregation sum over k on the PE: identity-weight matmuls
            # accumulate the Kj alpha-scaled [P, D] blocks into one PSUM tile
            pagg = psp.tile([P, D], f32, tag="pagg", bufs=2)
            for k in range(Kj):
                nc.tensor.matmul(
                    pagg[:],
                    id_sb[:],
                    bass.AP(Xg_all.tensor, Xg_all.offset + base + k * D,
                            [Xg_all.ap[0], [1, D]]),
                    start=(k == 0), stop=(k == Kj - 1),
                    skip_group_check=True)
            outs = outp.tile([P, D], f32, tag="outs")
            nc.vector.scalar_tensor_tensor(outs[:], pagg[:], 0.0,
                                           invatt_sb[:], Alu.bypass, Alu.mult)
            nc.vector.tensor_tensor(outs[:], outs[:], biasr_sb[:], Alu.add)
            nc.vector.scalar_tensor_tensor(outs[:], outs[:], SLOPE, outs[:],
                                           Alu.mult, Alu.max)
            outs_bf = outp.tile([P, D], f16, tag="outs_bf")
            nc.scalar.copy(outs_bf[:], outs[:])
            po = psp.tile([P, P], f16, tag="ptb", bufs=1)
            nc.tensor.transpose(po[:], outs_bf[:], id_sb[:])
            oT = outT[:, j * D:(j + 1) * D]
            nc.scalar.copy(oT, po[:])

            # W1 matvec for this block's genes, 4 genes per matmul
            for t0 in range(0, P, GB):
                wt1 = w1p.tile([P, GB * D], f16, tag="wt1")
                nc.sync.dma_start(
                    wt1[:], w1.ap()[:, (j * P + t0) * D:(j * P + t0 + GB) * D])
                for a in range(0, GB, 4):
                    t = t0 + a
                    g = j * P + t
                    nc.tensor.matmul(
                        psum_y[:], outT[:, j * D + t: j * D + t + 4],
                        wt1[:, a * D:(a + 4) * D],
                        start=(g == 0), stop=(g == PADG - 4),
                        skip_group_check=True)

        # ---------------- phase E: partial AllGather + MLP tail
        ycp = stg.tile([4, 4 * D], f32, tag="ycp")
        nc.scalar.copy(ycp[:], psum_y[:])
        ydg = stg.tile([1, 4 * D], f32, tag="ydg")
        diag = bass.AP(ycp.tensor, ycp.offset, [[ycp.ap[0][0] + D, 4], [1, D]])
        nc.sync.dma_start(ydg[:], diag)
        ysb = stg.tile([1, D], f32, tag="ysb")
        ydg_r = bass.AP(ydg.tensor, ydg.offset, [ydg.ap[0], [1, D], [D, 4]])
        nc.vector.tensor_reduce(ysb[:], ydg_r, axis=AxX, op=Alu.add)
        nc.sync.dma_start(yp_local.ap(), ysb[:])
        nc.gpsimd.collective_compute(
            "AllGather", Alu.bypass, replica_groups=rg,
            ins=[yp_local.ap()], outs=[yp_ag.ap()])
        ycols = stg.tile([D, C], f32, tag="ycols")
        yag_t = bass.AP(yp_ag, 0, [[1, D], [D, C]])
        nc.sync.dma_start(ycols[:], yag_t)
        ysum = stg.tile([D, 1], f32, tag="ysum")
        nc.vector.tensor_reduce(ysum[:], ycols[:], axis=AxX, op=Alu.add)
        nc.vector.tensor_scalar_mul(ysum[:], ysum[:], 1.0 / 256.0)
        nc.vector.tensor_tensor(ysum[:], ysum[:], b1_sb[:], Alu.add)
        nc.vector.tensor_scalar_max(ysum[:], ysum[:], 0.0)
        nc.vector.tensor_tensor(ysum[:], ysum[:], w2_sb[:], Alu.mult)
        pf = pyp.tile([1, 1], f32, tag="pf")
        nc.tensor.matmul(pf[:], ysum[:], ones_c[:], start=True, stop=True)
        fin = stg.tile([1, 1], f32, tag="fin")
        nc.scalar.copy(fin[:], pf[:])
        nc.vector.tensor_tensor(fin[:], fin[:], b2_sb[:], Alu.add)
        nc.sync.dma_start(y_out.ap(), fin[:])

    nc.compile()
    return nc


def _in_maps(prep):
    maps = []
    shared = dict(
        wl=prep["wl_eff"], wr=prep["wr_eff"], blr=prep["bl_eff"],
        brr=prep["br_eff"], ident=np.eye(P, dtype=F16),
        invatt=prep["inv_att"], biasr=prep["bias_r"], b1c=prep["b1c"],
        w2c=prep["w2c"], b2c=prep["b2c"])
    for c in range(C):
        m = dict(shared)
        m["x_bd8"] = prep["x_bd8"][c]
        m["w_in"] = prep["w_in_hk"][c]
        m["b_in_T"] = prep["b_in_T"][c]
        m["maskf"] = prep["mask"][c]
        m["idx16"] = prep["idx16"][c]
        m["w1"] = prep["w1_h"][c]
        maps.append(m)
    return maps


_CACHE = {}


def _get_kernel(prep):
    key = (tuple(prep["K"]), prep["p_pos"])
    if key not in _CACHE:
        _CACHE[key] = _build(prep["K"], prep["sumK"], prep["S"],
                             prep["p_pos"])
    return _CACHE[key]


def kernel(**inputs):
    from concourse.bass_utils import run_bass_kernel_spmd
    prep = _host_prep(inputs)
    nc = _get_kernel(prep)
    res = run_bass_kernel_spmd(nc, _in_maps(prep), list(range(C)))
    return res.results[0]["y_out"].reshape(1).astype(np.float32)


# debug helper: run through the multi-core simulator instead of hardware
def kernel_sim(**inputs):
    from concourse.bass_interp import MultiCoreSim
    prep = _host_prep(inputs)
    nc = _get_kernel(prep)
    sim = MultiCoreSim(nc, num_cores=C, trace=False,
                       require_finite=False, require_nnan=False)
    for c in range(C):
        for name, arr in _in_maps(prep)[c].items():
            sim.cores[c].tensor(name)[:] = arr
    sim.simulate(check_with_hw=False)
    return np.array(sim.cores[0].tensor("y_out")).reshape(1).astype(np.float32)


# revision 16
# speedup vs baseline: 1.1636x; 1.1636x over previous
"""Trainium2 Bass kernel for nn_GATv2_43499428773954.

Sharding (8 NeuronCores, SPMD):
  - Genes (nodes) are sharded across cores by a degree-sorted snake deal:
    every core owns 625 genes (padded to 640 = 5 tiles of 128). Per-gene
    weights W_in and the flatten-dense W1 are read only by the owning core,
    so the dominant HBM traffic is fully sharded.
  - All heavy tensors are f16 on device (W_in, W1, h, xl table, gather
    payloads, attention elementwise) with fp32 PSUM accumulation and an
    fp32 softmax; this halves HBM traffic and runs the PE/DVE at 2-4x
    the fp32 rate. Tolerance is 2e-2 rel; f16 keeps us ~1e-3.
  - GATv2 edge attention uses a padded-CSR layout: per tile of 128
    destination nodes, K_j slots each (K_j = tile max degree). Source
    features come from dma_gather row-gathers (split over 4 SWDGE queues)
    out of an AllGathered f16 xl table in DRAM; all 5 tiles' gathers are
    issued upfront so the gather stream never idles.
  - att is folded into W_l/W_r columns host-side and the d axis permuted
    so att>0 columns come first; then att . leaky(xl[s]+xr[t]) =
    sum_P max(v,.2v) + sum_N min(v,.2v) -> two in-place DVE ops + reduce.
  - h = relu(einsum('gi,gid->gd')) runs on the PE as one matmul per gene
    PAIR: lhsT = the pair's W_in rows stacked on the contraction axis,
    rhs = a block-diagonal [128,2] x column pair (built host-side), which
    yields hT tiles directly in PSUM.
  - Output MLP: per-gene W1 row-block matvecs accumulate into a [1,128]
    PSUM tile on the PE; AllGather of per-core partials + tiny MLP tail.
"""

import math
import numpy as np
F16 = np.float16

G = 5000
IN = 64
D = 128
E = 240000
C = 8
P = 128
GPC = G // C              # 625
TPC = (GPC + P - 1) // P  # 5
PADG = TPC * P            # 640
SLOPE = 0.2
MASKVAL = -1.0e5
NQ = 4                    # SWDGE queues for gather desc-gen
VCH = 32                  # V-buffer chunk (slots of K) for the e pass


# ----------------------------------------------------------------- host prep

def _host_prep(inputs):
    f32 = np.float32
    ei = np.asarray(inputs["edge_index"])
    loop = np.arange(G, dtype=ei.dtype)
    src = np.concatenate([ei[0], loop]).astype(np.int64)
    dst = np.concatenate([ei[1], loop]).astype(np.int64)
    deg = np.bincount(dst, minlength=G)

    order = np.argsort(-deg, kind="stable")
    perm = np.empty((C, GPC), dtype=np.int64)
    k = np.arange(G) % (2 * C)
    core_of_rank = np.where(k < C, k, 2 * C - 1 - k)
    for c in range(C):
        perm[c] = order[core_of_rank == c]
    newid = np.empty(G, dtype=np.int64)
    for c in range(C):
        newid[perm[c]] = c * PADG + np.arange(GPC)

    K = []
    for j in range(TPC):
        lo, hi = j * P, min((j + 1) * P, GPC)
        mx = max(int(deg[perm[c][lo:hi]].max()) for c in range(C))
        K.append(int(math.ceil(mx / 4) * 4))
    sumK = int(np.sum(K))
    S = sumK * P

    # CSR by destination, sources in new-id space
    order_e = np.argsort(dst, kind="stable")
    src_s = newid[src[order_e]]
    dst_s = dst[order_e]
    starts = np.searchsorted(dst_s, np.arange(G))
    ends = np.searchsorted(dst_s, np.arange(G) + 1)

    srcs_hat = np.zeros((C, S), dtype=np.int16)
    mask = np.zeros((C, P, sumK), dtype=f32)
    for c in range(C):
        off = 0
        koff = 0
        for j in range(TPC):
            Kj = K[j]
            for p in range(P):
                pos = j * P + p
                if pos < GPC:
                    g = perm[c][pos]
                    s0, s1 = starts[g], ends[g]
                    nd = s1 - s0
                    srcs_hat[c, off + np.arange(nd) * P + p] = src_s[s0:s1]
                else:
                    nd = 0
                mask[c, p, koff + nd:koff + Kj] = MASKVAL
            off += Kj * P
            koff += Kj
    mask = np.ascontiguousarray(mask)

    # wrap gather indices: logical idx i -> [i % 16, i // 16], replicated
    # into each Q7 core pair's 16-partition slice
    wrapped = srcs_hat.reshape(C, S // 16, 16).transpose(0, 2, 1)
    idx16 = np.tile(wrapped, (1, 8, 1))

    att = np.asarray(inputs["att"], dtype=f32)
    pi = np.argsort(att <= 0, kind="stable")
    p_pos = int((att > 0).sum())
    W_l = np.asarray(inputs["W_l"], f32)
    b_l = np.asarray(inputs["b_l"], f32)
    W_r = np.asarray(inputs["W_r"], f32)
    b_r = np.asarray(inputs["b_r"], f32)
    # xl/xr path carries a x16 scale for fp16 range; inv_att absorbs it
    wl_eff = (W_l * att[None, :] * 16.0)[:, pi].astype(F16)
    bl_eff = (b_l * att * 16.0)[pi].reshape(1, D).astype(F16)
    wr_eff = (W_r * att[None, :] * 16.0)[:, pi].astype(F16)
    br_eff = (b_r * att * 16.0)[pi].reshape(1, D).astype(F16)
    inv_att = np.tile((1.0 / att / 16.0)[pi][None, :], (P, 1)).astype(f32)
    bias_r = np.tile(np.asarray(inputs["bias"], f32)[pi][None, :], (P, 1))

    x = np.asarray(inputs["x"], f32)
    W_in = np.asarray(inputs["W_in"], f32)
    b_in = np.asarray(inputs["b_in"], f32)
    # phase-A layouts. Pair cc of tile j covers local genes (2cc, 2cc+1);
    # contraction index k = m*64 + i.
    #  w_in_hk[c][j, k, cc, d] = W_in[gene(j, 2cc + k//64), k%64, d]
    #  x_bd8[c][k, (j*16+q)*8 + 2p+m] = x[gene(j, 8q+2p+m), k%64] (k//64==m)
    NQUAD = P // 8  # 16 quads per tile
    w_in_hk = np.zeros((C, TPC, P, 64, D), F16)
    x_bd8 = np.zeros((C, P, TPC * NQUAD * 8), F16)
    b_in_T = np.zeros((C, D, TPC * P), f32)
    for c in range(C):
        xp = np.zeros((PADG, IN), f32)
        xp[:GPC] = x[perm[c]]
        wp = np.zeros((PADG, IN, D), f32)
        wp[:GPC] = W_in[perm[c]]
        bp = np.zeros((PADG, D), f32)
        bp[:GPC] = b_in[perm[c]]
        b_in_T[c] = bp.reshape(TPC * P, D).T
        for j in range(TPC):
            gj = wp[j * P:(j + 1) * P].reshape(64, 2, IN, D)  # [cc, m, i, d]
            # x16 scale for fp16 range; unscaled on-chip in the PSUM->hT copy
            w_in_hk[c, j] = (gj.transpose(1, 2, 0, 3).reshape(P, 64, D)
                             * 16.0).astype(F16)
            for q in range(NQUAD):
                for p4 in range(4):
                    for m in range(2):
                        gl = 8 * q + 2 * p4 + m
                        col = (j * NQUAD + q) * 8 + 2 * p4 + m
                        x_bd8[c, m * 64:(m + 1) * 64, col] = xp[j * P + gl]

    # W1 in [d, (gene, n)] layout: w1_h[c][d, g*128+n] = W1r[gene g][pi[d], n]
    W1r = np.asarray(inputs["W1"], f32).reshape(G, D, D)
    w1_h = np.zeros((C, D, PADG * D), F16)
    for c in range(C):
        t = np.zeros((PADG, D, D), f32)
        # x256 scale for fp16 range; unscaled in the MLP tail
        t[:GPC] = W1r[perm[c]][:, pi, :] * 256.0
        w1_h[c] = t.transpose(1, 0, 2).reshape(D, PADG * D)

    b1c = np.asarray(inputs["b1"], f32).reshape(D, 1).copy()
    w2c = np.asarray(inputs["W2"], f32).reshape(D, 1).copy()
    b2c = np.asarray(inputs["b2"], f32).reshape(1, 1).copy()

    return dict(K=K, sumK=sumK, S=S, p_pos=p_pos, idx16=idx16, mask=mask,
                wl_eff=wl_eff, bl_eff=bl_eff, wr_eff=wr_eff, br_eff=br_eff,
                inv_att=inv_att, bias_r=bias_r, x_bd8=x_bd8, w_in_hk=w_in_hk,
                b_in_T=b_in_T, w1_h=w1_h, b1c=b1c, w2c=w2c, b2c=b2c)


# -------------------------------------------------------------- bass builder

def _build(K, sumK, S, p_pos):
    import concourse.bass as bass
    import concourse.bacc as bacc
    import concourse.tile as tile
    from concourse import mybir
    from contextlib import ExitStack

    f32 = mybir.dt.float32
    f16 = mybir.dt.float16
    i16 = mybir.dt.int16
    Alu = mybir.AluOpType
    Act = mybir.ActivationFunctionType
    AxX = mybir.AxisListType.X

    nc = bacc.Bacc("TRN2", target_bir_lowering=False, debug=False,
                   enable_asserts=False, num_devices=C, num_swdge_queues=NQ)

    din = {}
    def inp(name, shape, dtype=f32):
        din[name] = nc.dram_tensor(name, list(shape), dtype,
                                   kind="ExternalInput")
        return din[name]

    inp("x_bd8", [P, TPC * 16 * 8], f16)
    w_in = inp("w_in", [TPC, P, 64, D], f16)
    inp("b_in_T", [D, TPC * P])
    inp("wl", [D, D], f16)
    inp("wr", [D, D], f16)
    inp("blr", [1, D], f16)
    inp("brr", [1, D], f16)
    inp("ident", [P, P], f16)
    inp("invatt", [P, D])
    inp("biasr", [P, D])
    inp("maskf", [P, sumK])
    inp("idx16", [P, S // 16], i16)
    w1 = inp("w1", [D, PADG * D], f16)
    inp("b1c", [D, 1])
    inp("w2c", [D, 1])
    inp("b2c", [1, 1])
    y_out = nc.dram_tensor("y_out", [1, 1], f32, kind="ExternalOutput")

    xl_local = nc.dram_tensor("xl_local", [PADG, D], f16)
    xl_ag = nc.dram_tensor("xl_ag", [C * PADG, D], f16, addr_space="Shared")
    yp_local = nc.dram_tensor("yp_local", [1, D], f32)
    yp_ag = nc.dram_tensor("yp_ag", [C, D], f32, addr_space="Shared")

    rg = [list(range(C))]

    with tile.TileContext(nc) as tc, ExitStack() as ctx:
        cp = ctx.enter_context(tc.tile_pool(name="const", bufs=1))

        def load(name, shape, dtype=f32):
            t = cp.tile(list(shape), dtype, tag=name)
            nc.sync.dma_start(t[:], din[name].ap())
            return t

        xbd_sb = load("x_bd8", [P, TPC * 16 * 8], f16)
        binT_sb = load("b_in_T", [D, TPC * P])
        wl_sb = load("wl", [D, D], f16)
        wr_sb = load("wr", [D, D], f16)
        blr_sb = load("blr", [1, D], f16)
        brr_sb = load("brr", [1, D], f16)
        id_sb = load("ident", [P, P], f16)
        invatt_sb = load("invatt", [P, D])
        biasr_sb = load("biasr", [P, D])
        mask_sb = load("maskf", [P, sumK])
        idx_sb = load("idx16", [P, S // 16], i16)
        b1_sb = load("b1c", [D, 1])
        w2_sb = load("w2c", [D, 1])
        b2_sb = load("b2c", [1, 1])
        ones_r = cp.tile([1, D], f16, tag="ones_r")
        nc.vector.memset(ones_r[:], 1.0)
        ones_c = cp.tile([D, 1], f32, tag="ones_c")
        nc.vector.memset(ones_c[:], 1.0)

        hT = cp.tile([P, TPC * D], f16, tag="hT")
        xr_sb = cp.tile([P, TPC * D], f16, tag="xr")
        outT = cp.tile([P, TPC * D], f16, tag="outT")
        e_sb = cp.tile([P, sumK], f32, tag="e")
        # one big gather target holding all 5 tiles' gathered xl rows
        Xg_all = cp.tile([P, sumK * D], f16, tag="xg_all")

        psp = ctx.enter_context(tc.tile_pool(name="ps", bufs=4, space="PSUM"))
        stg = ctx.enter_context(tc.tile_pool(name="stg", bufs=2))

        # ---------------- phase A: hT = relu(einsum(x, W_in) + b_in).T
        CB = 32  # pair-chunks per DMA (1MB f16)
        with tc.tile_pool(name="win", bufs=3) as winp:
            for j in range(TPC):
                ph = psp.tile([P, P], f32, tag="ph", bufs=2)
                for c0 in range(0, 64, CB):
                    wt = winp.tile([P, CB * D], f16, tag="wt")
                    srcap = bass.AP(w_in, (j * P * 64 + c0) * D,
                                    [[64 * D, P], [1, CB * D]])
                    nc.sync.dma_start(wt[:], srcap)
                    for cc in range(CB):
                        cgl = c0 + cc
                        kcol = (j * 16 + cgl // 4) * 8 + (cgl % 4) * 2
                        nc.tensor.matmul(
                            ph[:, 2 * cgl:2 * cgl + 2],
                            wt[:, cc * D:(cc + 1) * D],
                            xbd_sb[:, kcol:kcol + 2],
                            start=True, stop=True)
                hsl = hT[:, j * D:(j + 1) * D]
                nc.vector.scalar_tensor_tensor(
                    hsl, ph[:], 1.0 / 16.0, binT_sb[:, j * D:(j + 1) * D],
                    Alu.mult, Alu.add)
                nc.vector.tensor_scalar_max(hsl, hsl, 0.0)
                # phase B for this tile: xr, xl, write xl tile to DRAM
                sl = slice(j * D, (j + 1) * D)
                pr = psp.tile([P, P], f32, tag="ph", bufs=2)
                nc.tensor.matmul(pr[:], hT[:, sl], wr_sb[:],
                                 start=True, stop=False)
                nc.tensor.matmul(pr[:], ones_r[:], brr_sb[:],
                                 start=False, stop=True)
                nc.scalar.copy(xr_sb[:, sl], pr[:])
                pl = psp.tile([P, P], f32, tag="ph", bufs=2)
                nc.tensor.matmul(pl[:], hT[:, sl], wl_sb[:],
                                 start=True, stop=False)
                nc.tensor.matmul(pl[:], ones_r[:], blr_sb[:],
                                 start=False, stop=True)
                xl_tmp = stg.tile([P, P], f16, tag="xl_tmp")
                nc.scalar.copy(xl_tmp[:], pl[:])
                nc.sync.dma_start(xl_local[j * P:(j + 1) * P, :], xl_tmp[:])

        nc.gpsimd.collective_compute(
            "AllGather", Alu.bypass, replica_groups=rg,
            ins=[xl_local.ap()], outs=[xl_ag.ap()])

        # ---------------- phase C prologue: issue ALL tiles' gathers upfront
        koffs = [int(np.sum(K[:j])) for j in range(TPC + 1)]
        for j in range(TPC):
            Kj = K[j]
            off = koffs[j] * P
            base = koffs[j] * D  # column offset into Xg_all
            ksplit = [Kj * q // NQ for q in range(NQ + 1)]
            for q in range(NQ):
                k0, k1 = ksplit[q], ksplit[q + 1]
                if k1 == k0:
                    continue
                n = (k1 - k0) * P
                gq = bass.AP(Xg_all.tensor, Xg_all.offset + base + k0 * D,
                             [Xg_all.ap[0], [D, k1 - k0], [1, D]])
                nc.gpsimd.dma_gather(
                    out_ap=gq, in_ap=xl_ag.ap(),
                    idxs_ap=idx_sb[:, (off + k0 * P) // 16:(off + k1 * P) // 16],
                    num_idxs=n, num_idxs_reg=n, elem_size=D,
                    single_packet=False, queue_num=q)

        # ---------------- phase C+D: attention blocks + W1 matvec
        vp = ctx.enter_context(tc.tile_pool(name="v", bufs=2))
        outp = ctx.enter_context(tc.tile_pool(name="outp", bufs=2))
        smp = ctx.enter_context(tc.tile_pool(name="sm", bufs=4))
        w1p = ctx.enter_context(tc.tile_pool(name="w1p", bufs=14))
        pyp = ctx.enter_context(tc.tile_pool(name="py", bufs=1, space="PSUM"))
        # 4-gene-per-matmul W1 accumulator: quad matmul outT[:,4].T @
        # [W1_g0|..|W1_g3] -> [4, 512]; only the 4 diagonal 128-blocks are
        # meaningful, and they accumulate sum_g out_g @ W1_g across quads.
        psum_y = pyp.tile([4, 4 * D], f32, tag="psum_y")

        GB = 16  # genes per W1 DMA (512KB f16)
        for j in range(TPC):
            Kj = K[j]
            koff = koffs[j]
            base = koff * D
            xg_j = bass.AP(Xg_all.tensor, Xg_all.offset + base,
                           [Xg_all.ap[0], [D, Kj], [1, D]])
            xr_j = xr_sb[:, j * D:(j + 1) * D]

            # e pass in V-chunks of VCH k-slots
            for k0 in range(0, Kj, VCH):
                kw = min(VCH, Kj - k0)
                V = vp.tile([P, VCH * D], f16, tag="v")
                Va = bass.AP(V.tensor, V.offset, [V.ap[0], [D, kw], [1, D]])
                Xc = bass.AP(Xg_all.tensor, Xg_all.offset + base + k0 * D,
                             [Xg_all.ap[0], [D, kw], [1, D]])
                xr_b = bass.AP(xr_j.tensor, xr_j.offset,
                               [xr_j.ap[0], [0, kw], xr_j.ap[1]])
                nc.vector.scalar_tensor_tensor(Va, Xc, 0.0, xr_b,
                                               Alu.bypass, Alu.add)
                VP = bass.AP(V.tensor, V.offset, [V.ap[0], [D, kw], [1, p_pos]])
                VN = bass.AP(V.tensor, V.offset + p_pos,
                             [V.ap[0], [D, kw], [1, D - p_pos]])
                nc.vector.scalar_tensor_tensor(VP, VP, SLOPE, VP,
                                               Alu.mult, Alu.max)
                nc.vector.scalar_tensor_tensor(VN, VN, SLOPE, VN,
                                               Alu.mult, Alu.min)
                e_c = e_sb[:, koff + k0:koff + k0 + kw]
                nc.vector.tensor_reduce(e_c, Va, axis=AxX, op=Alu.add)

            e_sl = e_sb[:, koff:koff + Kj]
            nc.vector.tensor_tensor(e_sl, e_sl,
                                    mask_sb[:, koff:koff + Kj], Alu.add)
            m = smp.tile([P, 1], f32, tag="m")
            nc.vector.tensor_reduce(m[:], e_sl, axis=AxX, op=Alu.max)
            negm = smp.tile([P, 1], f32, tag="negm")
            nc.vector.tensor_scalar_mul(negm[:], m[:], -1.0 / 16.0)
            nc.scalar.activation(e_sl, e_sl, Act.Exp, bias=negm[:],
                                 scale=1.0 / 16.0)
            z = smp.tile([P, 1], f32, tag="z")
            nc.vector.tensor_reduce(z[:], e_sl, axis=AxX, op=Alu.add)
            zr = smp.tile([P, 1], f32, tag="zr")
            nc.vector.reciprocal(zr[:], z[:])
            nc.vector.tensor_scalar_mul(e_sl, e_sl, zr[:])   # alpha
            al_bf = smp.tile([P, sumK], f16, tag="al_bf", bufs=2)
            nc.scalar.copy(al_bf[:, :Kj], e_sl)
            al_b = bass.AP(al_bf.tensor, al_bf.offset,
                           [al_bf.ap[0], [1, Kj], [0, D]])
            nc.vector.tensor_tensor(xg_j, xg_j, al_b, Alu.mult)
            # aggregation sum over k on the PE: identity-weight matmuls
            # accumulate the Kj alpha-scaled [P, D] blocks into one PSUM tile
            pagg = psp.tile([P, D], f32, tag="pagg", bufs=2)
            for k in range(Kj):
                nc.tensor.matmul(
                    pagg[:],
                    id_sb[:],
                    bass.AP(Xg_all.tensor, Xg_all.offset + base + k * D,
                            [Xg_all.ap[0], [1, D]]),
                    start=(k == 0), stop=(k == Kj - 1),
                    skip_group_check=True)
            outs = outp.tile([P, D], f32, tag="outs")
            nc.vector.scalar_tensor_tensor(outs[:], pagg[:], 0.0,
                                           invatt_sb[:], Alu.bypass, Alu.mult)
            nc.vector.tensor_tensor(outs[:], outs[:], biasr_sb[:], Alu.add)
            nc.vector.scalar_tensor_tensor(outs[:], outs[:], SLOPE, outs[:],
                                           Alu.mult, Alu.max)
            outs_bf = outp.tile([P, D], f16, tag="outs_bf")
            nc.scalar.copy(outs_bf[:], outs[:])
            po = psp.tile([P, P], f16, tag="ptb", bufs=1)
            nc.tensor.transpose(po[:], outs_bf[:], id_sb[:])
            oT = outT[:, j * D:(j + 1) * D]
            nc.scalar.copy(oT, po[:])

            # W1 matvec for this block's genes, 4 genes per matmul
            for t0 in range(0, P, GB):
                wt1 = w1p.tile([P, GB * D], f16, tag="wt1")
                nc.sync.dma_start(
                    wt1[:], w1.ap()[:, (j * P + t0) * D:(j * P + t0 + GB) * D])
                for a in range(0, GB, 4):
                    t = t0 + a
                    g = j * P + t
                    nc.tensor.matmul(
                        psum_y[:], outT[:, j * D + t: j * D + t + 4],
                        wt1[:, a * D:(a + 4) * D],
                        start=(g == 0), stop=(g == PADG - 4),
                        skip_group_check=True)

        # ---------------- phase E: partial AllGather + MLP tail
        ycp = stg.tile([4, 4 * D], f32, tag="ycp")
        nc.scalar.copy(ycp[:], psum_y[:])
        ydg = stg.tile([1, 4 * D], f32, tag="ydg")
        diag = bass.AP(ycp.tensor, ycp.offset, [[ycp.ap[0][0] + D, 4], [1, D]])
        nc.sync.dma_start(ydg[:], diag)
        ysb = stg.tile([1, D], f32, tag="ysb")
        ydg_r = bass.AP(ydg.tensor, ydg.offset, [ydg.ap[0], [1, D], [D, 4]])
        nc.vector.tensor_reduce(ysb[:], ydg_r, axis=AxX, op=Alu.add)
        nc.sync.dma_start(yp_local.ap(), ysb[:])
        nc.gpsimd.collective_compute(
            "AllGather", Alu.bypass, replica_groups=rg,
            ins=[yp_local.ap()], outs=[yp_ag.ap()])
        ycols = stg.tile([D, C], f32, tag="ycols")
        yag_t = bass.AP(yp_ag, 0, [[1, D], [D, C]])
        nc.sync.dma_start(ycols[:], yag_t)
        ysum = stg.tile([D, 1], f32, tag="ysum")
        nc.vector.tensor_reduce(ysum[:], ycols[:], axis=AxX, op=Alu.add)
        nc.vector.tensor_scalar_mul(ysum[:], ysum[:], 1.0 / 256.0)
        nc.vector.tensor_tensor(ysum[:], ysum[:], b1_sb[:], Alu.add)
        nc.vector.tensor_scalar_max(ysum[:], ysum[:], 0.0)
        nc.vector.tensor_tensor(ysum[:], ysum[:], w2_sb[:], Alu.mult)
        pf = pyp.tile([1, 1], f32, tag="pf")
        nc.tensor.matmul(pf[:], ysum[:], ones_c[:], start=True, stop=True)
        fin = stg.tile([1, 1], f32, tag="fin")
        nc.scalar.copy(fin[:], pf[:])
        nc.vector.tensor_tensor(fin[:], fin[:], b2_sb[:], Alu.add)
        nc.sync.dma_start(y_out.ap(), fin[:])

    nc.compile()
    return nc


def _in_maps(prep):
    maps = []
    shared = dict(
        wl=prep["wl_eff"], wr=prep["wr_eff"], blr=prep["bl_eff"],
        brr=prep["br_eff"], ident=np.eye(P, dtype=F16),
        invatt=prep["inv_att"], biasr=prep["bias_r"], b1c=prep["b1c"],
        w2c=prep["w2c"], b2c=prep["b2c"])
    for c in range(C):
        m = dict(shared)
        m["x_bd8"] = prep["x_bd8"][c]
        m["w_in"] = prep["w_in_hk"][c]
        m["b_in_T"] = prep["b_in_T"][c]
        m["maskf"] = prep["mask"][c]
        m["idx16"] = prep["idx16"][c]
        m["w1"] = prep["w1_h"][c]
        maps.append(m)
    return maps


_CACHE = {}


def _get_kernel(prep):
    key = (tuple(prep["K"]), prep["p_pos"])
    if key not in _CACHE:
        _CACHE[key] = _build(prep["K"], prep["sumK"], prep["S"],
                             prep["p_pos"])
    return _CACHE[key]


def kernel(**inputs):
    from concourse.bass_utils import run_bass_kernel_spmd
    prep = _host_prep(inputs)
    nc = _get_kernel(prep)
    res = run_bass_kernel_spmd(nc, _in_maps(prep), list(range(C)))
    return res.results[0]["y_out"].reshape(1).astype(np.float32)


# debug helper: run through the multi-core simulator instead of hardware
def kernel_sim(**inputs):
    from concourse.bass_interp import MultiCoreSim
    prep = _host_prep(inputs)
    nc = _get_kernel(prep)
    sim = MultiCoreSim(nc, num_cores=C, trace=False,
                       require_finite=False, require_nnan=False)
    for c in range(C):
        for name, arr in _in_maps(prep)[c].items():
            sim.cores[c].tensor(name)[:] = arr
    sim.simulate(check_with_hw=False)
    return np.array(sim.cores[0].tensor("y_out")).reshape(1).astype(np.float32)
